# revision 1
# baseline (speedup 1.0000x reference)
"""Trainium2 Bass kernel for nn_ConvexSimilarityHash.

Reference computation (B=16, T=4096, E=1024, HALF=2048, WIN=15):
  x_t    = x * taper[None, :, None]
  c0     = x_t[..., 0];  r = |c0| + 1e-6
  start  = where(c0 >= 0, 0, pi)
  v      = clip(x_t[..., 1:] / r, -1+1e-6, 1-1e-6)
  phases = start + sum_e arcsin(v)                       # (B, T)
  tdraw  = causal triangular MA of c0 over WIN lags      # (B, T)
  thash  = tanh(silu(phases @ cW1.T + cb1) @ cW2.T + cb2)
  tdiff  = tanh(silu(tdraw  @ pW1.T + pb1) @ pW2.T + pb2)
  out    = stack([thash, tdiff], -1)                     # (B, HALF, 2)

Sharding (8 cores): T-sharded elementwise + layer-1 partial matmuls
(PSUM accumulation), AllReduce of the (2, HALF, B) pre-activations,
feature-sharded layer 2 (each core computes HALF/8 output features).

arcsin(w) = atan(w * rsqrt(1 - w^2)); rsqrt via Abs_reciprocal_sqrt.
HW arctan is full-range (verified on silicon). w = clip(s*x, +-c) is done
as s * clip(x, +-c/s), and w^2 is additionally clamped to <= c^2 so that
1 - w^2 never goes negative from rounding.
"""
from contextlib import ExitStack

import numpy as np

import concourse.bacc as bacc
import concourse.tile as tile
from concourse import mybir
from concourse.bass_utils import run_bass_kernel_spmd

AF = mybir.ActivationFunctionType
ALU = mybir.AluOpType
F32 = mybir.dt.float32

B, T, E = 16, 4096, 1024
HALF = T // 2
WIN = 15
EPS = 1e-6
CLIP = 1.0 - 1e-6
CLIP2 = float(np.float32(CLIP) * np.float32(CLIP))
NCORE = 8
TLOC = T // NCORE          # 512 timesteps per core
NCH = TLOC // 128          # 4 chunks of 128 timesteps
OSL = HALF // NCORE        # 256 output features per core
NOT = OSL // 128           # 2 output tiles per compressor
NHT = HALF // 128          # 16 hidden tiles
PI = float(np.pi)


def build_nc(sim_safe=False, act_square_mod=1000000, debug_taps=False, no_cc=False):
    nc = bacc.Bacc("TRN2", target_bir_lowering=False, debug=False,
                   num_devices=NCORE)

    xs_h = nc.dram_tensor("xs", [B, TLOC, E], F32, kind="ExternalInput")
    c0t_h = nc.dram_tensor("c0t", [128, (NCH + 1) * B], F32, kind="ExternalInput")
    tap_h = nc.dram_tensor("tap", [128, NCH], F32, kind="ExternalInput")
    citap_h = nc.dram_tensor("citap", [128, NCH], F32, kind="ExternalInput")
    invn_h = nc.dram_tensor("invn", [128, NCH], F32, kind="ExternalInput")
    convA_h = nc.dram_tensor("convA", [128, 128], F32, kind="ExternalInput")
    convB_h = nc.dram_tensor("convB", [128, 128], F32, kind="ExternalInput")
    w1c_h = nc.dram_tensor("w1c", [TLOC, HALF], F32, kind="ExternalInput")
    w1p_h = nc.dram_tensor("w1p", [TLOC, HALF], F32, kind="ExternalInput")
    w2c_h = nc.dram_tensor("w2c", [HALF, OSL], F32, kind="ExternalInput")
    w2p_h = nc.dram_tensor("w2p", [HALF, OSL], F32, kind="ExternalInput")
    b1big_h = nc.dram_tensor("b1big", [128, 2 * NHT * B], F32, kind="ExternalInput")
    b2c_h = nc.dram_tensor("b2c", [128, 2 * NOT], F32, kind="ExternalInput")
    out_h = nc.dram_tensor("out", [2, NOT, 128, B], F32, kind="ExternalOutput")

    cc_in = nc.dram_tensor("cc_in", [128, 2 * NHT * B], F32)
    cc_out = nc.dram_tensor("cc_out", [128, 2 * NHT * B], F32, addr_space="Shared")

    dbg = {}
    if debug_taps:
        for nm, shp in (("dbg_ph", [128, NCH * B]), ("dbg_td", [128, NCH * B]),
                        ("dbg_pre", [128, 2 * NHT * B]),
                        ("dbg_red", [128, 2 * NHT * B]),
                        ("dbg_h1", [128, 2 * NHT * B]),
                        ("dbg_s", [128, NCH * B]), ("dbg_hi", [128, NCH * B])):
            dbg[nm] = nc.dram_tensor(nm, shp, F32, kind="ExternalOutput")

    with tile.TileContext(nc) as tc, ExitStack() as ctx:
        consts = ctx.enter_context(tc.tile_pool(name="consts", bufs=1))
        w1pool = ctx.enter_context(tc.tile_pool(name="w1", bufs=2))
        w2pool = ctx.enter_context(tc.tile_pool(name="w2", bufs=1))
        xpool = ctx.enter_context(tc.tile_pool(name="x", bufs=3))
        wcyp = ctx.enter_context(tc.tile_pool(name="wcy", bufs=B + 2))
        s2p = ctx.enter_context(tc.tile_pool(name="s2p", bufs=3))
        small = ctx.enter_context(tc.tile_pool(name="small", bufs=2))
        tailp = ctx.enter_context(tc.tile_pool(name="tail", bufs=1))
        psum = ctx.enter_context(tc.tile_pool(name="psum", bufs=2, space="PSUM"))
        psc = ctx.enter_context(tc.tile_pool(name="psc", bufs=2, space="PSUM"))
        ps2 = ctx.enter_context(tc.tile_pool(name="ps2", bufs=2, space="PSUM"))

        def load_const(h, shape):
            t = consts.tile(shape, F32, tag=h.name)
            nc.sync.dma_start(out=t, in_=h.ap())
            return t

        c0t = load_const(c0t_h, [128, (NCH + 1) * B])
        tap = load_const(tap_h, [128, NCH])
        citap = load_const(citap_h, [128, NCH])
        invn = load_const(invn_h, [128, NCH])
        convA = load_const(convA_h, [128, 128])
        convB = load_const(convB_h, [128, 128])
        b1big = load_const(b1big_h, [128, 2 * NHT * B])
        b2c = load_const(b2c_h, [128, 2 * NOT])

        bm1 = consts.tile([128, 1], F32, tag="bm1")
        nc.vector.memset(bm1, -1.0)

        # SBUF accumulator for layer-1 pre-activations (PSUM accumulation
        # across interleaved groups is broken on HW; single-shot matmuls
        # into disjoint PSUM columns + SBUF adds are exact)
        sacc = consts.tile([128, 2 * NHT * B], F32, tag="sacc")

        xs = xs_h.ap()
        w2t = {}

        for j in range(NCH):
            c0_prev = c0t[:, j * B:(j + 1) * B]
            c0_loc = c0t[:, (j + 1) * B:(j + 2) * B]

            ra = small.tile([128, B], F32, tag="ra")
            nc.scalar.activation(out=ra, in_=c0_loc, func=AF.Abs)
            r = small.tile([128, B], F32, tag="r")
            nc.vector.tensor_scalar(out=r, in0=ra, scalar1=EPS, scalar2=None,
                                    op0=ALU.add)
            rr = small.tile([128, B], F32, tag="rr")
            nc.vector.reciprocal(out=rr, in_=r)
            s = small.tile([128, B], F32, tag="s")
            nc.vector.tensor_scalar(out=s, in0=rr, scalar1=tap[:, j:j + 1],
                                    scalar2=None, op0=ALU.mult)
            ss = small.tile([128, B], F32, tag="ss")
            nc.vector.tensor_tensor(out=ss, in0=s, in1=s, op=ALU.mult)
            hi = small.tile([128, B], F32, tag="hi")
            nc.vector.tensor_scalar(out=hi, in0=r, scalar1=citap[:, j:j + 1],
                                    scalar2=None, op0=ALU.mult)
            lo = small.tile([128, B], F32, tag="lo")
            nc.vector.tensor_scalar(out=lo, in0=hi, scalar1=-1.0, scalar2=None,
                                    op0=ALU.mult)
            startT = small.tile([128, B], F32, tag="startT")
            nc.vector.tensor_scalar(out=startT, in0=c0_loc, scalar1=0.0,
                                    scalar2=PI, op0=ALU.is_lt, op1=ALU.mult)

            # tdiff_raw via banded conv matmul + invnorm scaling
            pc = psc.tile([128, B], F32, tag="pconv")
            nc.tensor.matmul(pc, lhsT=convA, rhs=c0_loc, start=True, stop=False)
            nc.tensor.matmul(pc, lhsT=convB, rhs=c0_prev, start=False, stop=True)
            tdraw = small.tile([128, B], F32, tag="tdraw")
            nc.vector.tensor_scalar(out=tdraw, in0=pc, scalar1=invn[:, j:j + 1],
                                    scalar2=None, op0=ALU.mult)

            phacc = small.tile([128, B], F32, tag="phacc")

            # --- per-b heavy tiles: everything except atan ---
            ytiles = []
            for b in range(B):
                s_col = s[:, b:b + 1]
                xt = xpool.tile([128, E], F32, tag="xt")
                nc.sync.dma_start(out=xt, in_=xs[b, j * 128:(j + 1) * 128, :])
                wc = wcyp.tile([128, E], F32, tag="wcy")
                nc.vector.tensor_scalar(out=wc, in0=xt, scalar1=lo[:, b:b + 1],
                                        scalar2=hi[:, b:b + 1],
                                        op0=ALU.max, op1=ALU.min)
                s2 = s2p.tile([128, E], F32, tag="s2")
                if (j * B + b) % act_square_mod == 0:
                    # ACT square path: s2 = (s*wc)^2, then clamp to CLIP2
                    nc.scalar.activation(out=s2, in_=wc, func=AF.Square,
                                         scale=s_col)
                    nc.vector.tensor_scalar(out=s2, in0=s2, scalar1=CLIP2,
                                            scalar2=None, op0=ALU.min)
                else:
                    # DVE square path: s2 = wc^2 * s^2 (fused), clamp
                    nc.vector.tensor_tensor(out=s2, in0=wc, in1=wc, op=ALU.mult)
                    nc.vector.tensor_scalar(out=s2, in0=s2,
                                            scalar1=ss[:, b:b + 1],
                                            scalar2=CLIP2,
                                            op0=ALU.mult, op1=ALU.min)
                # q = 1/sqrt(1 - s2), in place over s2
                if sim_safe:
                    nc.vector.tensor_scalar(out=s2, in0=s2, scalar1=-1.0,
                                            scalar2=1.0, op0=ALU.mult, op1=ALU.add)
                    nc.scalar.activation(out=s2, in_=s2, func=AF.Sqrt)
                    q = s2p.tile([128, E], F32, tag="qsim")
                    nc.vector.reciprocal(out=q, in_=s2)
                else:
                    nc.scalar.activation(out=s2, in_=s2,
                                         func=AF.Abs_reciprocal_sqrt,
                                         bias=bm1[:, 0:1])
                    q = s2
                # y = wc * q, in place over wc
                nc.vector.tensor_tensor(out=wc, in0=wc, in1=q, op=ALU.mult)
                ytiles.append(wc)

            # --- atan batch (one ACT table set) ---
            for b in range(B):
                y = ytiles[b]
                nc.scalar.activation(out=y[:, 1:E], in_=y[:, 1:E],
                                     func=AF.Arctan, scale=s[:, b:b + 1],
                                     accum_out=phacc[:, b:b + 1])

            phasesT = small.tile([128, B], F32, tag="phasesT")
            nc.vector.tensor_tensor(out=phasesT, in0=phacc, in1=startT,
                                    op=ALU.add)

            if debug_taps:
                nc.sync.dma_start(out=dbg["dbg_ph"].ap()[:, j * B:(j + 1) * B],
                                  in_=phasesT)
                nc.sync.dma_start(out=dbg["dbg_td"].ap()[:, j * B:(j + 1) * B],
                                  in_=tdraw)
                nc.sync.dma_start(out=dbg["dbg_s"].ap()[:, j * B:(j + 1) * B],
                                  in_=s)
                nc.sync.dma_start(out=dbg["dbg_hi"].ap()[:, j * B:(j + 1) * B],
                                  in_=hi)

            # layer-1 partial matmuls: single-shot groups into one PSUM tile
            pl1 = psum.tile([128, 2 * NHT * B], F32, tag="pl1")
            for c, (w1h, rhs) in enumerate(((w1c_h, phasesT), (w1p_h, tdraw))):
                w1tile = w1pool.tile([128, HALF], F32, tag=f"w1_{c}")
                nc.sync.dma_start(out=w1tile,
                                  in_=w1h.ap()[j * 128:(j + 1) * 128, :])
                for ht in range(NHT):
                    nc.tensor.matmul(
                        pl1[:, (c * NHT + ht) * B:(c * NHT + ht + 1) * B],
                        lhsT=w1tile[:, ht * 128:(ht + 1) * 128],
                        rhs=rhs, start=True, stop=True,
                        skip_group_check=True)
            if j == 0:
                nc.vector.tensor_copy(out=sacc, in_=pl1)
            else:
                nc.vector.tensor_tensor(out=sacc, in0=sacc, in1=pl1, op=ALU.add)

            if j == NCH - 2:
                # prefetch layer-2 weights late (keeps SBUF headroom earlier)
                for c, w2h in ((0, w2c_h), (1, w2p_h)):
                    for kk in range(NHT):
                        t = w2pool.tile([128, OSL], F32, tag=f"w2_{c}_{kk}")
                        nc.sync.dma_start(
                            out=t, in_=w2h.ap()[kk * 128:(kk + 1) * 128, :])
                        w2t[(c, kk)] = t

        # ---- tail: bias, AllReduce, silu, layer 2, tanh ----
        pre = tailp.tile([128, 2 * NHT * B], F32, tag="pre")
        nc.vector.tensor_tensor(out=pre, in0=sacc, in1=b1big, op=ALU.add)
        nc.sync.dma_start(out=cc_in.ap(), in_=pre)
        red = tailp.tile([128, 2 * NHT * B], F32, tag="red")
        if no_cc:
            # timing-sim variant: skip the collective, keep equivalent DMAs
            nc.sync.dma_start(out=red, in_=cc_in.ap())
        else:
            nc.gpsimd.collective_compute(
                "AllReduce", ALU.add, replica_groups=[list(range(NCORE))],
                ins=[cc_in.ap()], outs=[cc_out.ap()])
            nc.sync.dma_start(out=red, in_=cc_out.ap())
        h1 = tailp.tile([128, 2 * NHT * B], F32, tag="h1")
        nc.scalar.activation(out=h1, in_=red, func=AF.Silu)

        if debug_taps:
            nc.sync.dma_start(out=dbg["dbg_pre"].ap(), in_=pre)
            nc.sync.dma_start(out=dbg["dbg_red"].ap(), in_=red)
            nc.sync.dma_start(out=dbg["dbg_h1"].ap(), in_=h1)

        for c in range(2):
            for ot in range(NOT):
                p2 = ps2.tile([128, B], F32, tag="p2")
                for kk in range(NHT):
                    nc.tensor.matmul(
                        p2, lhsT=w2t[(c, kk)][:, ot * 128:(ot + 1) * 128],
                        rhs=h1[:, (c * NHT + kk) * B:(c * NHT + kk + 1) * B],
                        start=(kk == 0), stop=(kk == NHT - 1))
                ot_sb = small.tile([128, B], F32, tag="ot_sb")
                nc.scalar.activation(out=ot_sb, in_=p2, func=AF.Tanh,
                                     bias=b2c[:, (c * NOT + ot):(c * NOT + ot) + 1])
                nc.sync.dma_start(out=out_h.ap()[c, ot, :, :], in_=ot_sb)

    nc.compile()
    return nc


def host_prepare(x, taper, cW1, cb1, cW2, cb2, pW1, pb1, pW2, pb2):
    """Build the 8 per-core input maps (numpy only)."""
    x = np.ascontiguousarray(np.asarray(x), dtype=np.float32)
    taper = np.asarray(taper, dtype=np.float32)
    cW1, cW2, pW1, pW2 = (np.asarray(a, np.float32) for a in (cW1, cW2, pW1, pW2))
    cb1, cb2, pb1, pb2 = (np.asarray(a, np.float32) for a in (cb1, cb2, pb1, pb2))

    c0_full = x[:, :, 0] * taper[None, :]           # (B, T)
    c0_pad = np.concatenate([np.zeros((B, 128), np.float32), c0_full], axis=1)
    i = np.arange(T, dtype=np.float32)
    sN = np.minimum(i, float(WIN))
    norm = sN * (sN + 1.0) * 0.5
    invnorm_full = np.where(norm > 0, 1.0 / np.maximum(norm, 1.0), 0.0).astype(np.float32)
    with np.errstate(divide="ignore"):
        citap_full = np.where(taper > 0, CLIP / taper, np.inf).astype(np.float32)

    pidx = np.arange(128)
    d_a = pidx[None, :] - pidx[:, None]             # p - p'
    convA = np.where((d_a >= 1) & (d_a <= WIN), d_a, 0).astype(np.float32)
    d_b = d_a + 128
    convB = np.where((d_b >= 1) & (d_b <= WIN), d_b, 0).astype(np.float32)

    in_maps = []
    for k in range(NCORE):
        t0 = k * TLOC
        tsl = slice(t0, t0 + TLOC)
        xs = np.ascontiguousarray(x[:, tsl, :])

        # tapered c0 with halo chunk in front: [128, (NCH+1), B] -> [128, .]
        blk = c0_pad[:, t0:t0 + 128 * (NCH + 1)]              # (B, 640)
        c0t = np.ascontiguousarray(
            blk.reshape(B, NCH + 1, 128).transpose(2, 1, 0).reshape(128, -1))

        tloc = t0 + np.arange(TLOC)
        tapm = np.ascontiguousarray(taper[tloc].reshape(NCH, 128).T)
        citap = np.ascontiguousarray(citap_full[tloc].reshape(NCH, 128).T)
        invn = np.ascontiguousarray(invnorm_full[tloc].reshape(NCH, 128).T)

        w1c = np.ascontiguousarray(cW1[:, tsl].T)
        w1p = np.ascontiguousarray(pW1[:, tsl].T)
        osl = slice(k * OSL, (k + 1) * OSL)
        w2c = np.ascontiguousarray(cW2[osl, :].T)
        w2p = np.ascontiguousarray(pW2[osl, :].T)

        b1big = np.empty((128, 2, NHT, B), np.float32)
        for c, b1 in enumerate((cb1, pb1)):
            b1m = b1.reshape(NHT, 128).T                      # [p, ht]
            b1big[:, c, :, :] = b1m[:, :, None]
        # each core adds the bias before the AllReduce -> divide by NCORE
        # (exact in fp32: NCORE is a power of two)
        b1big = np.ascontiguousarray(b1big.reshape(128, -1)) / np.float32(NCORE)

        b2cols = np.empty((128, 2, NOT), np.float32)
        for c, b2 in enumerate((cb2, pb2)):
            b2cols[:, c, :] = b2[osl].reshape(NOT, 128).T
        b2cols = np.ascontiguousarray(b2cols.reshape(128, -1))

        in_maps.append(dict(
            xs=xs, c0t=c0t, tap=tapm, citap=citap, invn=invn,
            convA=convA, convB=convB,
            w1c=w1c, w1p=w1p, w2c=w2c, w2p=w2p,
            b1big=b1big, b2c=b2cols))
    return in_maps


def assemble_output(results):
    out = np.empty((B, HALF, 2), np.float32)
    for k, r in enumerate(results):
        o = np.asarray(r["out"]).reshape(2, NOT, 128, B)      # [c, ot, p, b]
        for c in range(2):
            for ot in range(NOT):
                out[:, k * OSL + ot * 128:k * OSL + (ot + 1) * 128, c] = o[c, ot].T
    return out


_NC_CACHE = {}


def _get_nc(**kw):
    key = tuple(sorted(kw.items()))
    if key not in _NC_CACHE:
        _NC_CACHE[key] = build_nc(**kw)
    return _NC_CACHE[key]


def run(inputs, trace=False, **build_kw):
    nc = _get_nc(**build_kw)
    in_maps = host_prepare(**inputs)
    res = run_bass_kernel_spmd(nc, in_maps, core_ids=list(range(NCORE)),
                               trace=trace)
    return assemble_output(res.results), res


def kernel(**inputs):
    out, _ = run(inputs)
    return out



# revision 2
# speedup vs baseline: 1.0179x; 1.0179x over previous
"""Trainium2 Bass kernel for nn_ConvexSimilarityHash.

Reference computation (B=16, T=4096, E=1024, HALF=2048, WIN=15):
  x_t    = x * taper[None, :, None]
  c0     = x_t[..., 0];  r = |c0| + 1e-6
  start  = where(c0 >= 0, 0, pi)
  v      = clip(x_t[..., 1:] / r, -1+1e-6, 1-1e-6)
  phases = start + sum_e arcsin(v)                       # (B, T)
  tdraw  = causal triangular MA of c0 over WIN lags      # (B, T)
  thash  = tanh(silu(phases @ cW1.T + cb1) @ cW2.T + cb2)
  tdiff  = tanh(silu(tdraw  @ pW1.T + pb1) @ pW2.T + pb2)
  out    = stack([thash, tdiff], -1)                     # (B, HALF, 2)

Sharding (8 cores): T-sharded elementwise + layer-1 partial matmuls,
AllReduce of the (2, HALF, B) pre-activations, feature-sharded layer 2
(each core computes HALF/8 output features).

arcsin(w) = atan(w * rsqrt(1 - w^2)); rsqrt via Abs_reciprocal_sqrt with
scale=-1, bias=+1 (u = |1 - w^2| computed inside ACT in fp32).

fp16 element pipeline: x is DMA'd as fp16; w = clip(s*x, +-1) is clipped
AFTER scaling so saturated elements pin to exactly 1.0 (fp16-exact);
then s2 = w^2 = 1.0 exactly and u = 0 -> q = inf -> atan(+-inf) = +-pi/2,
matching arcsin(+-(1-1e-6)) to 1.4e-3 per element.
"""
from contextlib import ExitStack

import numpy as np

import concourse.bacc as bacc
import concourse.tile as tile
from concourse import mybir
from concourse.bass_utils import run_bass_kernel_spmd

AF = mybir.ActivationFunctionType
ALU = mybir.AluOpType
F32 = mybir.dt.float32
F16 = mybir.dt.float16

B, T, E = 16, 4096, 1024
HALF = T // 2
WIN = 15
EPS = 1e-6
PI = float(np.pi)
NCORE = 8
TLOC = T // NCORE          # 512 timesteps per core
NCH = TLOC // 128          # 4 chunks of 128 timesteps
OSL = HALF // NCORE        # 256 output features per core
NOT = OSL // 128           # 2 output tiles per compressor
NHT = HALF // 128          # 16 hidden tiles


def build_nc(debug_taps=False, no_cc=False):
    nc = bacc.Bacc("TRN2", target_bir_lowering=False, debug=False,
                   num_devices=NCORE)

    xs_h = nc.dram_tensor("xs", [B, TLOC, E], F16, kind="ExternalInput")
    c0t_h = nc.dram_tensor("c0t", [128, (NCH + 1) * B], F32, kind="ExternalInput")
    tap_h = nc.dram_tensor("tap", [128, NCH], F32, kind="ExternalInput")
    invn_h = nc.dram_tensor("invn", [128, NCH], F32, kind="ExternalInput")
    convA_h = nc.dram_tensor("convA", [128, 128], F32, kind="ExternalInput")
    convB_h = nc.dram_tensor("convB", [128, 128], F32, kind="ExternalInput")
    w1c_h = nc.dram_tensor("w1c", [TLOC, HALF], F32, kind="ExternalInput")
    w1p_h = nc.dram_tensor("w1p", [TLOC, HALF], F32, kind="ExternalInput")
    w2c_h = nc.dram_tensor("w2c", [HALF, OSL], F32, kind="ExternalInput")
    w2p_h = nc.dram_tensor("w2p", [HALF, OSL], F32, kind="ExternalInput")
    b1big_h = nc.dram_tensor("b1big", [128, 2 * NHT * B], F32, kind="ExternalInput")
    b2c_h = nc.dram_tensor("b2c", [128, 2 * NOT], F32, kind="ExternalInput")
    out_h = nc.dram_tensor("out", [2, NOT, 128, B], F32, kind="ExternalOutput")

    cc_in = nc.dram_tensor("cc_in", [128, 2 * NHT * B], F32)
    cc_out = nc.dram_tensor("cc_out", [128, 2 * NHT * B], F32, addr_space="Shared")

    dbg = {}
    if debug_taps:
        for nm, shp in (("dbg_ph", [128, NCH * B]), ("dbg_td", [128, NCH * B]),
                        ("dbg_pre", [128, 2 * NHT * B]),
                        ("dbg_s", [128, NCH * B])):
            dbg[nm] = nc.dram_tensor(nm, shp, F32, kind="ExternalOutput")

    with tile.TileContext(nc) as tc, ExitStack() as ctx:
        consts = ctx.enter_context(tc.tile_pool(name="consts", bufs=1))
        w1pool = ctx.enter_context(tc.tile_pool(name="w1", bufs=2))
        w2pool = ctx.enter_context(tc.tile_pool(name="w2", bufs=1))
        xpool = ctx.enter_context(tc.tile_pool(name="x", bufs=4))
        wpool = ctx.enter_context(tc.tile_pool(name="wp", bufs=3))
        s2p = ctx.enter_context(tc.tile_pool(name="s2p", bufs=3))
        yp = ctx.enter_context(tc.tile_pool(name="yp", bufs=3))
        small = ctx.enter_context(tc.tile_pool(name="small", bufs=2))
        tailp = ctx.enter_context(tc.tile_pool(name="tail", bufs=1))
        psum = ctx.enter_context(tc.tile_pool(name="psum", bufs=2, space="PSUM"))
        psc = ctx.enter_context(tc.tile_pool(name="psc", bufs=2, space="PSUM"))
        ps2 = ctx.enter_context(tc.tile_pool(name="ps2", bufs=2, space="PSUM"))

        def load_const(h, shape):
            t = consts.tile(shape, F32, tag=h.name, name=h.name)
            nc.sync.dma_start(out=t, in_=h.ap())
            return t

        c0t = load_const(c0t_h, [128, (NCH + 1) * B])
        tap = load_const(tap_h, [128, NCH])
        invn = load_const(invn_h, [128, NCH])
        convA = load_const(convA_h, [128, 128])
        convB = load_const(convB_h, [128, 128])
        b1big = load_const(b1big_h, [128, 2 * NHT * B])
        b2c = load_const(b2c_h, [128, 2 * NOT])

        # SBUF accumulator for layer-1 pre-activations (PSUM accumulation
        # across interleaved groups is broken on HW; single-shot matmuls
        # into disjoint PSUM columns + SBUF adds are exact)
        sacc = consts.tile([128, 2 * NHT * B], F32, tag="sacc")

        xs = xs_h.ap()
        w2t = {}

        for j in range(NCH):
            c0_prev = c0t[:, j * B:(j + 1) * B]
            c0_loc = c0t[:, (j + 1) * B:(j + 2) * B]

            ra = small.tile([128, B], F32, tag="ra")
            nc.scalar.activation(out=ra, in_=c0_loc, func=AF.Abs)
            r = small.tile([128, B], F32, tag="r")
            nc.vector.tensor_scalar(out=r, in0=ra, scalar1=EPS, scalar2=None,
                                    op0=ALU.add)
            rr = small.tile([128, B], F32, tag="rr")
            nc.vector.reciprocal(out=rr, in_=r)
            s = small.tile([128, B], F32, tag="s")
            nc.vector.tensor_scalar(out=s, in0=rr, scalar1=tap[:, j:j + 1],
                                    scalar2=None, op0=ALU.mult)
            startT = small.tile([128, B], F32, tag="startT")
            nc.vector.tensor_scalar(out=startT, in0=c0_loc, scalar1=0.0,
                                    scalar2=PI, op0=ALU.is_lt, op1=ALU.mult)

            # tdiff_raw via banded conv matmul + invnorm scaling
            pc = psc.tile([128, B], F32, tag="pconv")
            nc.tensor.matmul(pc, lhsT=convA, rhs=c0_loc, start=True, stop=False)
            nc.tensor.matmul(pc, lhsT=convB, rhs=c0_prev, start=False, stop=True)
            tdraw = small.tile([128, B], F32, tag="tdraw")
            nc.vector.tensor_scalar(out=tdraw, in0=pc, scalar1=invn[:, j:j + 1],
                                    scalar2=None, op0=ALU.mult)

            phacc = small.tile([128, B], F32, tag="phacc")

            for b in range(B):
                s_col = s[:, b:b + 1]
                xt = xpool.tile([128, E], F16, tag="xt")
                nc.sync.dma_start(out=xt, in_=xs[b, j * 128:(j + 1) * 128, :])
                # w = clip(s*x, +-1): clipped values pin to exactly 1.0 (f16)
                w = wpool.tile([128, E], F16, tag="w")
                nc.vector.tensor_scalar(out=w, in0=xt, scalar1=s_col,
                                        scalar2=-1.0, op0=ALU.mult, op1=ALU.max)
                nc.vector.tensor_scalar(out=w, in0=w, scalar1=1.0,
                                        scalar2=None, op0=ALU.min)
                s2 = s2p.tile([128, E], F16, tag="s2")
                nc.vector.tensor_tensor(out=s2, in0=w, in1=w, op=ALU.mult)
                # q = 1/sqrt(|1 - s2|)  (u computed in fp32 inside ACT)
                q = s2p.tile([128, E], F16, tag="q")
                nc.scalar.activation(out=q, in_=s2, func=AF.Abs_reciprocal_sqrt,
                                     scale=-1.0, bias=1.0)
                y = yp.tile([128, E], F16, tag="y")
                nc.vector.tensor_tensor(out=y, in0=w, in1=q, op=ALU.mult)
                nc.scalar.activation(out=y[:, 1:E], in_=y[:, 1:E],
                                     func=AF.Arctan,
                                     accum_out=phacc[:, b:b + 1])

            phasesT = small.tile([128, B], F32, tag="phasesT")
            nc.vector.tensor_tensor(out=phasesT, in0=phacc, in1=startT,
                                    op=ALU.add)

            if debug_taps:
                nc.sync.dma_start(out=dbg["dbg_ph"].ap()[:, j * B:(j + 1) * B],
                                  in_=phasesT)
                nc.sync.dma_start(out=dbg["dbg_td"].ap()[:, j * B:(j + 1) * B],
                                  in_=tdraw)
                nc.sync.dma_start(out=dbg["dbg_s"].ap()[:, j * B:(j + 1) * B],
                                  in_=s)

            # layer-1 partial matmuls: single-shot groups into one PSUM tile
            pl1 = psum.tile([128, 2 * NHT * B], F32, tag="pl1")
            for c, (w1h, rhs) in enumerate(((w1c_h, phasesT), (w1p_h, tdraw))):
                w1tile = w1pool.tile([128, HALF], F32, tag=f"w1_{c}")
                nc.sync.dma_start(out=w1tile,
                                  in_=w1h.ap()[j * 128:(j + 1) * 128, :])
                for ht in range(NHT):
                    nc.tensor.matmul(
                        pl1[:, (c * NHT + ht) * B:(c * NHT + ht + 1) * B],
                        lhsT=w1tile[:, ht * 128:(ht + 1) * 128],
                        rhs=rhs, start=True, stop=True,
                        skip_group_check=True)
            if j == 0:
                nc.vector.tensor_copy(out=sacc, in_=pl1)
            else:
                nc.vector.tensor_tensor(out=sacc, in0=sacc, in1=pl1, op=ALU.add)

            if j == NCH - 2:
                # prefetch layer-2 weights late (keeps SBUF headroom earlier)
                for c, w2h in ((0, w2c_h), (1, w2p_h)):
                    for kk in range(NHT):
                        t = w2pool.tile([128, OSL], F32, tag=f"w2_{c}_{kk}",
                                        name=f"w2_{c}_{kk}")
                        nc.sync.dma_start(
                            out=t, in_=w2h.ap()[kk * 128:(kk + 1) * 128, :])
                        w2t[(c, kk)] = t

        # ---- tail: bias, AllReduce, silu, layer 2, tanh ----
        pre = tailp.tile([128, 2 * NHT * B], F32, tag="pre")
        nc.vector.tensor_tensor(out=pre, in0=sacc, in1=b1big, op=ALU.add)
        nc.sync.dma_start(out=cc_in.ap(), in_=pre)
        red = tailp.tile([128, 2 * NHT * B], F32, tag="red")
        if no_cc:
            # timing-sim variant: skip the collective, keep equivalent DMAs
            nc.sync.dma_start(out=red, in_=cc_in.ap())
        else:
            nc.gpsimd.collective_compute(
                "AllReduce", ALU.add, replica_groups=[list(range(NCORE))],
                ins=[cc_in.ap()], outs=[cc_out.ap()])
            nc.sync.dma_start(out=red, in_=cc_out.ap())
        h1 = tailp.tile([128, 2 * NHT * B], F32, tag="h1")
        nc.scalar.activation(out=h1, in_=red, func=AF.Silu)

        if debug_taps:
            nc.sync.dma_start(out=dbg["dbg_pre"].ap(), in_=pre)

        for c in range(2):
            for ot in range(NOT):
                p2 = ps2.tile([128, B], F32, tag="p2")
                for kk in range(NHT):
                    nc.tensor.matmul(
                        p2, lhsT=w2t[(c, kk)][:, ot * 128:(ot + 1) * 128],
                        rhs=h1[:, (c * NHT + kk) * B:(c * NHT + kk + 1) * B],
                        start=(kk == 0), stop=(kk == NHT - 1))
                ot_sb = small.tile([128, B], F32, tag="ot_sb")
                nc.scalar.activation(out=ot_sb, in_=p2, func=AF.Tanh,
                                     bias=b2c[:, (c * NOT + ot):(c * NOT + ot) + 1])
                nc.sync.dma_start(out=out_h.ap()[c, ot, :, :], in_=ot_sb)

    nc.compile()
    return nc


def host_prepare(x, taper, cW1, cb1, cW2, cb2, pW1, pb1, pW2, pb2):
    """Build the 8 per-core input maps (numpy only)."""
    x = np.ascontiguousarray(np.asarray(x), dtype=np.float32)
    taper = np.asarray(taper, dtype=np.float32)
    cW1, cW2, pW1, pW2 = (np.asarray(a, np.float32) for a in (cW1, cW2, pW1, pW2))
    cb1, cb2, pb1, pb2 = (np.asarray(a, np.float32) for a in (cb1, cb2, pb1, pb2))

    x16 = x.astype(np.float16)

    c0_full = x[:, :, 0] * taper[None, :]           # (B, T)
    c0_pad = np.concatenate([np.zeros((B, 128), np.float32), c0_full], axis=1)
    i = np.arange(T, dtype=np.float32)
    sN = np.minimum(i, float(WIN))
    norm = sN * (sN + 1.0) * 0.5
    invnorm_full = np.where(norm > 0, 1.0 / np.maximum(norm, 1.0), 0.0).astype(np.float32)

    pidx = np.arange(128)
    d_a = pidx[None, :] - pidx[:, None]             # p - p'
    convA = np.where((d_a >= 1) & (d_a <= WIN), d_a, 0).astype(np.float32)
    d_b = d_a + 128
    convB = np.where((d_b >= 1) & (d_b <= WIN), d_b, 0).astype(np.float32)

    in_maps = []
    for k in range(NCORE):
        t0 = k * TLOC
        tsl = slice(t0, t0 + TLOC)
        xs = np.ascontiguousarray(x16[:, tsl, :])

        # tapered c0 with halo chunk in front: [128, (NCH+1), B] -> [128, .]
        blk = c0_pad[:, t0:t0 + 128 * (NCH + 1)]              # (B, 640)
        c0t = np.ascontiguousarray(
            blk.reshape(B, NCH + 1, 128).transpose(2, 1, 0).reshape(128, -1))

        tloc = t0 + np.arange(TLOC)
        tapm = np.ascontiguousarray(taper[tloc].reshape(NCH, 128).T)
        invn = np.ascontiguousarray(invnorm_full[tloc].reshape(NCH, 128).T)

        w1c = np.ascontiguousarray(cW1[:, tsl].T)
        w1p = np.ascontiguousarray(pW1[:, tsl].T)
        osl = slice(k * OSL, (k + 1) * OSL)
        w2c = np.ascontiguousarray(cW2[osl, :].T)
        w2p = np.ascontiguousarray(pW2[osl, :].T)

        b1big = np.empty((128, 2, NHT, B), np.float32)
        for c, b1 in enumerate((cb1, pb1)):
            b1m = b1.reshape(NHT, 128).T                      # [p, ht]
            b1big[:, c, :, :] = b1m[:, :, None]
        # each core adds the bias before the AllReduce -> divide by NCORE
        # (exact in fp32: NCORE is a power of two)
        b1big = np.ascontiguousarray(b1big.reshape(128, -1)) / np.float32(NCORE)

        b2cols = np.empty((128, 2, NOT), np.float32)
        for c, b2 in enumerate((cb2, pb2)):
            b2cols[:, c, :] = b2[osl].reshape(NOT, 128).T
        b2cols = np.ascontiguousarray(b2cols.reshape(128, -1))

        in_maps.append(dict(
            xs=xs, c0t=c0t, tap=tapm, invn=invn,
            convA=convA, convB=convB,
            w1c=w1c, w1p=w1p, w2c=w2c, w2p=w2p,
            b1big=b1big, b2c=b2cols))
    return in_maps


def assemble_output(results):
    out = np.empty((B, HALF, 2), np.float32)
    for k, r in enumerate(results):
        o = np.asarray(r["out"]).reshape(2, NOT, 128, B)      # [c, ot, p, b]
        for c in range(2):
            for ot in range(NOT):
                out[:, k * OSL + ot * 128:k * OSL + (ot + 1) * 128, c] = o[c, ot].T
    return out


_NC_CACHE = {}


def _get_nc(**kw):
    key = tuple(sorted(kw.items()))
    if key not in _NC_CACHE:
        _NC_CACHE[key] = build_nc(**kw)
    return _NC_CACHE[key]


def run(inputs, trace=False, **build_kw):
    nc = _get_nc(**build_kw)
    in_maps = host_prepare(**inputs)
    res = run_bass_kernel_spmd(nc, in_maps, core_ids=list(range(NCORE)),
                               trace=trace)
    return assemble_output(res.results), res


def kernel(**inputs):
    out, _ = run(inputs)
    return out


# revision 14
# speedup vs baseline: 1.4303x; 1.4051x over previous
"""Trainium2 Bass kernel for nn_ConvexSimilarityHash.

Reference computation (B=16, T=4096, E=1024, HALF=2048, WIN=15):
  x_t    = x * taper[None, :, None]
  c0     = x_t[..., 0];  r = |c0| + 1e-6
  start  = where(c0 >= 0, 0, pi)
  v      = clip(x_t[..., 1:] / r, -1+1e-6, 1-1e-6)
  phases = start + sum_e arcsin(v)                       # (B, T)
  tdraw  = causal triangular MA of c0 over WIN lags      # (B, T)
  thash  = tanh(silu(phases @ cW1.T + cb1) @ cW2.T + cb2)
  tdiff  = tanh(silu(tdraw  @ pW1.T + pb1) @ pW2.T + pb2)
  out    = stack([thash, tdiff], -1)                     # (B, HALF, 2)

Sharding (8 cores): T-sharded elementwise + layer-1 partial matmuls,
AllReduce of the (2, HALF, B) pre-activations, feature-sharded layer 2.

arcsin(w) = atan(w * rsqrt(1 - w^2)), w = clip(s*x, +-1), s = taper/r.
Clipping AFTER scaling pins saturated elements to exactly 1.0 (fp16-
exact), so u = 1-w^2 = 0 -> q = inf -> atan(+-inf) = +-pi/2, matching
arcsin(+-(1-1e-6)) to 1.4e-3.

Engine split per 128-timestep chunk (16 batch rows, fp16 elementwise):
  DVE : w = clip(s*x,+-1) (2 tensor_scalar @4x), s2 = w*w (tt @2x),
        y = w*q (tt @2x)
  ACT : q = rsqrt(|1-s2|) batched 4 rows/op, atan batched 4 rows/op
        (f32 out, no accum) -- 2 table loads per chunk
  Pool: per-row accumulation of atan outputs via tensor_scalar accum_out
  PE  : layer-1/2 matmuls (fp16 weights), banded-conv tdiff
"""
from contextlib import ExitStack

import numpy as np

import concourse.bacc as bacc
import concourse.tile as tile
from concourse import mybir
from concourse.bass_utils import run_bass_kernel_spmd

AF = mybir.ActivationFunctionType
ALU = mybir.AluOpType
F32 = mybir.dt.float32
F16 = mybir.dt.float16

B, T, E = 16, 4096, 1024
HALF = T // 2
WIN = 15
EPS = 1e-6
PI = float(np.pi)
NCORE = 8
TLOC = T // NCORE          # 512 timesteps per core
NCH = TLOC // 128          # 4 chunks of 128 timesteps
OSL = HALF // NCORE        # 256 output features per core
NOT = OSL // 128           # 2 output tiles per compressor
NHT = HALF // 128          # 16 hidden tiles
G = 4                      # batch rows per ACT op
NG = B // G                # groups per chunk


SQ_POOL = (1, 3)    # square row indices (within group) on Pool
YM_POOL = ()          # y-mult group indices on the Pool engine
YM_POOL_LAST = (2, 3)  # same, for the last chunk (shortens the drain)


def build_nc(debug_taps=False, no_cc=False):
    nc = bacc.Bacc("TRN2", target_bir_lowering=False, debug=False,
                   num_devices=NCORE)

    xs_h = nc.dram_tensor("xs", [B, TLOC, E], F16, kind="ExternalInput")
    c0t_h = nc.dram_tensor("c0t", [128, (NCH + 1) * B], F32, kind="ExternalInput")
    tap_h = nc.dram_tensor("tap", [128, NCH], F32, kind="ExternalInput")
    invn_h = nc.dram_tensor("invn", [128, NCH], F32, kind="ExternalInput")
    convA_h = nc.dram_tensor("convA", [128, 128], F32, kind="ExternalInput")
    convB_h = nc.dram_tensor("convB", [128, 128], F32, kind="ExternalInput")
    w1c_h = nc.dram_tensor("w1c", [TLOC, HALF], F16, kind="ExternalInput")
    w1p_h = nc.dram_tensor("w1p", [TLOC, HALF], F16, kind="ExternalInput")
    w2c_h = nc.dram_tensor("w2c", [HALF, OSL], F16, kind="ExternalInput")
    w2p_h = nc.dram_tensor("w2p", [HALF, OSL], F16, kind="ExternalInput")
    b1big_h = nc.dram_tensor("b1big", [128, 2 * NHT * B], F32, kind="ExternalInput")
    b2c_h = nc.dram_tensor("b2c", [128, 2 * NOT], F32, kind="ExternalInput")
    out_h = nc.dram_tensor("out", [2, NOT, 128, B], F32, kind="ExternalOutput")

    cc_in = nc.dram_tensor("cc_in", [128, 2 * NHT * B], F32)
    cc_out = nc.dram_tensor("cc_out", [128, 2 * NHT * B], F32, addr_space="Shared")

    dbg = {}
    if debug_taps:
        for nm, shp in (("dbg_ph", [128, NCH * B]), ("dbg_td", [128, NCH * B]),
                        ("dbg_pre", [128, 2 * NHT * B])):
            dbg[nm] = nc.dram_tensor(nm, shp, F32, kind="ExternalOutput")

    with tile.TileContext(nc) as tc, ExitStack() as ctx:
        consts = ctx.enter_context(tc.tile_pool(name="consts", bufs=1))
        w1pool = ctx.enter_context(tc.tile_pool(name="w1", bufs=2))
        w2pool = ctx.enter_context(tc.tile_pool(name="w2", bufs=1))
        xpool = ctx.enter_context(tc.tile_pool(name="x", bufs=3))
        wpool = ctx.enter_context(tc.tile_pool(name="wp", bufs=B + 4))
        s2p = ctx.enter_context(tc.tile_pool(name="s2p", bufs=NG + 5))
        y32p = ctx.enter_context(tc.tile_pool(name="y32p", bufs=2))
        small = ctx.enter_context(tc.tile_pool(name="small", bufs=2))
        tailp = ctx.enter_context(tc.tile_pool(name="tail", bufs=1))
        psum = ctx.enter_context(tc.tile_pool(name="psum", bufs=2, space="PSUM"))
        psc = ctx.enter_context(tc.tile_pool(name="psc", bufs=2, space="PSUM"))
        ps2 = ctx.enter_context(tc.tile_pool(name="ps2", bufs=2, space="PSUM"))

        def load_const(h, shape, dt=F32):
            t = consts.tile(shape, dt, tag=h.name, name=h.name)
            nc.sync.dma_start(out=t, in_=h.ap())
            return t

        c0t = load_const(c0t_h, [128, (NCH + 1) * B])
        tap = load_const(tap_h, [128, NCH])
        invn = load_const(invn_h, [128, NCH])
        convA = load_const(convA_h, [128, 128])
        convB = load_const(convB_h, [128, 128])
        b1big = load_const(b1big_h, [128, 2 * NHT * B])
        b2c = load_const(b2c_h, [128, 2 * NOT])

        # SBUF accumulator for layer-1 pre-activations (PSUM accumulation
        # across interleaved groups is broken on HW; single-shot matmuls
        # into disjoint PSUM columns + SBUF adds are exact)
        sacc = consts.tile([128, 2 * NHT * B], F32, tag="sacc")

        xs = xs_h.ap()
        w2t = {}
        acc_eng = [nc.vector, nc.vector, nc.vector, nc.vector]
        stash = {}

        def emit_smalls(j):
            c0_prev = c0t[:, j * B:(j + 1) * B]
            c0_loc = c0t[:, (j + 1) * B:(j + 2) * B]
            ra = small.tile([128, B], F32, tag="ra", name=f"ra{j}")
            nc.scalar.activation(out=ra, in_=c0_loc, func=AF.Abs)
            r = small.tile([128, B], F32, tag="r", name=f"r{j}")
            nc.vector.tensor_scalar(out=r, in0=ra, scalar1=EPS, scalar2=None,
                                    op0=ALU.add)
            rr = small.tile([128, B], F32, tag="rr", name=f"rr{j}")
            nc.vector.reciprocal(out=rr, in_=r)
            s = small.tile([128, B], F32, tag="s", name=f"s{j}")
            nc.vector.tensor_scalar(out=s, in0=rr, scalar1=tap[:, j:j + 1],
                                    scalar2=None, op0=ALU.mult)
            startT = small.tile([128, B], F32, tag="startT", name=f"startT{j}")
            nc.vector.tensor_scalar(out=startT, in0=c0_loc, scalar1=0.0,
                                    scalar2=PI, op0=ALU.is_lt, op1=ALU.mult)
            pc = psc.tile([128, B], F32, tag="pconv", name=f"pc{j}")
            nc.tensor.matmul(pc, lhsT=convA, rhs=c0_loc, start=True, stop=False)
            nc.tensor.matmul(pc, lhsT=convB, rhs=c0_prev, start=False, stop=True)
            tdraw = small.tile([128, B], F16, tag="tdraw", name=f"tdraw{j}")
            nc.vector.tensor_scalar(out=tdraw, in0=pc, scalar1=invn[:, j:j + 1],
                                    scalar2=None, op0=ALU.mult)
            return s, startT, tdraw

        def emit_A(j, s):
            """clips + squares per group, rsqrt batched per group (fp16)."""
            wt, s2g = {}, {}
            for g in range(NG):
                s2 = s2p.tile([128, G * E], F16, tag="s2", name=f"s2_{j}_{g}")
                s2g[g] = s2
                for i in range(G):
                    b = g * G + i
                    xt = xpool.tile([128, E], F16, tag="xt", name=f"xt{j}_{b}")
                    nc.sync.dma_start(out=xt,
                                      in_=xs[b, j * 128:(j + 1) * 128, :])
                    # w = clip(s*x, +-1): saturated pins to exactly 1.0 (f16)
                    w = wpool.tile([128, E], F16, tag="w", name=f"w_{j}_{b}")
                    nc.vector.tensor_scalar(out=w, in0=xt,
                                            scalar1=s[:, b:b + 1],
                                            scalar2=-1.0,
                                            op0=ALU.mult, op1=ALU.max)
                    nc.vector.tensor_scalar(out=w, in0=w, scalar1=1.0,
                                            scalar2=None, op0=ALU.min)
                    wt[b] = w
                    sq_eng = nc.gpsimd if i in SQ_POOL else nc.vector
                    sq_eng.tensor_tensor(out=s2[:, i * E:(i + 1) * E],
                                         in0=w, in1=w, op=ALU.mult)
                # q = 1/sqrt(|1 - s2|), in place (u formed in fp32 inside ACT)
                nc.scalar.activation(out=s2, in_=s2,
                                     func=AF.Abs_reciprocal_sqrt,
                                     scale=-1.0, bias=1.0)
            return wt, s2g

        def emit_B(j, wt, s2g, startT, tdraw):
            """y = w*q in place over q, batched atan (f32), split accums."""
            phacc = small.tile([128, B], F32, tag="phacc", name=f"phacc{j}")
            for g in range(NG):
                s2 = s2g[g]
                for i in range(G):
                    b = g * G + i
                    ymp = YM_POOL_LAST if j == NCH - 1 else YM_POOL
                    ym_eng = nc.gpsimd if i in ymp else nc.vector
                    ym_eng.tensor_tensor(out=s2[:, i * E:(i + 1) * E],
                                         in0=wt[b],
                                         in1=s2[:, i * E:(i + 1) * E],
                                         op=ALU.mult)
                y32 = y32p.tile([128, G * E], F32, tag="y32",
                                name=f"y32_{j}_{g}")
                nc.scalar.activation(out=y32, in_=s2, func=AF.Arctan)
                for i in range(G):
                    b = g * G + i
                    sl = y32[:, i * E + 1:(i + 1) * E]
                    acc_eng[i].tensor_scalar(out=sl, in0=sl, scalar1=1.0,
                                             scalar2=0.0, op0=ALU.mult,
                                             op1=ALU.add,
                                             accum_out=phacc[:, b:b + 1])
            phasesT = small.tile([128, B], F16, tag="phasesT", name=f"ph{j}")
            nc.vector.tensor_tensor(out=phasesT, in0=phacc, in1=startT,
                                    op=ALU.add)
            if debug_taps:
                nc.sync.dma_start(out=dbg["dbg_ph"].ap()[:, j * B:(j + 1) * B],
                                  in_=phasesT)
                nc.sync.dma_start(out=dbg["dbg_td"].ap()[:, j * B:(j + 1) * B],
                                  in_=tdraw)
            # layer-1 partial matmuls: single-shot groups into one PSUM tile
            pl1 = psum.tile([128, 2 * NHT * B], F32, tag="pl1", name=f"pl1{j}")
            for c, (w1h, rhs) in enumerate(((w1c_h, phasesT), (w1p_h, tdraw))):
                w1tile = w1pool.tile([128, HALF], F16, tag=f"w1_{c}",
                                     name=f"w1_{c}_{j}")
                nc.sync.dma_start(out=w1tile,
                                  in_=w1h.ap()[j * 128:(j + 1) * 128, :])
                for ht in range(NHT):
                    nc.tensor.matmul(
                        pl1[:, (c * NHT + ht) * B:(c * NHT + ht + 1) * B],
                        lhsT=w1tile[:, ht * 128:(ht + 1) * 128],
                        rhs=rhs, start=True, stop=True,
                        skip_group_check=True)
            if j == 0:
                # fold the (pre-divided) layer-1 bias in at init
                nc.vector.tensor_tensor(out=sacc, in0=pl1, in1=b1big,
                                        op=ALU.add)
            else:
                nc.vector.tensor_tensor(out=sacc, in0=sacc, in1=pl1,
                                        op=ALU.add)

        # software-pipelined j loop: B(j-1) emitted before A(j)
        for j in range(NCH + 1):
            if j <= 1:
                pass
            if j < NCH:
                stash[j] = emit_smalls(j)
            if j > 0:
                sp, stp, tdp = stash.pop(j - 1)
                wt, s2g = stash.pop((j - 1, "A"))
                emit_B(j - 1, wt, s2g, stp, tdp)
            if j < NCH:
                stash[(j, "A")] = emit_A(j, stash[j][0])
            if j == NCH - 1:
                # prefetch layer-2 weights late (keeps SBUF headroom earlier)
                for c, w2h in ((0, w2c_h), (1, w2p_h)):
                    for kk in range(NHT):
                        t = w2pool.tile([128, OSL], F16, tag=f"w2_{c}_{kk}",
                                        name=f"w2_{c}_{kk}")
                        nc.sync.dma_start(
                            out=t, in_=w2h.ap()[kk * 128:(kk + 1) * 128, :])
                        w2t[(c, kk)] = t

        # ---- tail: AllReduce, silu, layer 2, tanh ----
        nc.sync.dma_start(out=cc_in.ap(), in_=sacc)
        red = tailp.tile([128, 2 * NHT * B], F32, tag="red")
        if no_cc:
            # timing-sim variant: skip the collective, keep equivalent DMAs
            nc.sync.dma_start(out=red, in_=cc_in.ap())
        else:
            nc.gpsimd.collective_compute(
                "AllReduce", ALU.add, replica_groups=[list(range(NCORE))],
                ins=[cc_in.ap()], outs=[cc_out.ap()])
            nc.sync.dma_start(out=red, in_=cc_out.ap())
        h1 = tailp.tile([128, 2 * NHT * B], F16, tag="h1")
        nc.scalar.activation(out=h1, in_=red, func=AF.Silu)
        if debug_taps:
            nc.sync.dma_start(out=dbg["dbg_pre"].ap(), in_=sacc)
        for c in range(2):
            for ot in range(NOT):
                p2 = ps2.tile([128, B], F32, tag="p2", name=f"p2_{c}_{ot}")
                for kk in range(NHT):
                    nc.tensor.matmul(
                        p2, lhsT=w2t[(c, kk)][:, ot * 128:(ot + 1) * 128],
                        rhs=h1[:, (c * NHT + kk) * B:(c * NHT + kk + 1) * B],
                        start=(kk == 0), stop=(kk == NHT - 1))
                ot_sb = small.tile([128, B], F32, tag="ot_sb",
                                   name=f"ot_{c}_{ot}")
                nc.scalar.activation(out=ot_sb, in_=p2, func=AF.Tanh,
                                     bias=b2c[:, (c * NOT + ot):(c * NOT + ot) + 1])
                nc.sync.dma_start(out=out_h.ap()[c, ot, :, :], in_=ot_sb)

    nc.compile()
    return nc


def host_prepare(x, taper, cW1, cb1, cW2, cb2, pW1, pb1, pW2, pb2):
    """Build the 8 per-core input maps (numpy only)."""
    x = np.ascontiguousarray(np.asarray(x), dtype=np.float32)
    taper = np.asarray(taper, dtype=np.float32)
    cW1, cW2, pW1, pW2 = (np.asarray(a, np.float32) for a in (cW1, cW2, pW1, pW2))
    cb1, cb2, pb1, pb2 = (np.asarray(a, np.float32) for a in (cb1, cb2, pb1, pb2))

    x16 = x.astype(np.float16)

    c0_full = x[:, :, 0] * taper[None, :]           # (B, T)
    c0_pad = np.concatenate([np.zeros((B, 128), np.float32), c0_full], axis=1)
    i = np.arange(T, dtype=np.float32)
    sN = np.minimum(i, float(WIN))
    norm = sN * (sN + 1.0) * 0.5
    invnorm_full = np.where(norm > 0, 1.0 / np.maximum(norm, 1.0), 0.0).astype(np.float32)

    pidx = np.arange(128)
    d_a = pidx[None, :] - pidx[:, None]             # p - p'
    convA = np.where((d_a >= 1) & (d_a <= WIN), d_a, 0).astype(np.float32)
    d_b = d_a + 128
    convB = np.where((d_b >= 1) & (d_b <= WIN), d_b, 0).astype(np.float32)

    in_maps = []
    for k in range(NCORE):
        t0 = k * TLOC
        tsl = slice(t0, t0 + TLOC)
        xs = np.ascontiguousarray(x16[:, tsl, :])

        # tapered c0 with halo chunk in front: [128, (NCH+1), B] -> [128, .]
        blk = c0_pad[:, t0:t0 + 128 * (NCH + 1)]              # (B, 640)
        c0t = np.ascontiguousarray(
            blk.reshape(B, NCH + 1, 128).transpose(2, 1, 0).reshape(128, -1))

        tloc = t0 + np.arange(TLOC)
        tapm = np.ascontiguousarray(taper[tloc].reshape(NCH, 128).T)
        invn = np.ascontiguousarray(invnorm_full[tloc].reshape(NCH, 128).T)

        w1c = np.ascontiguousarray(cW1[:, tsl].T.astype(np.float16))
        w1p = np.ascontiguousarray(pW1[:, tsl].T.astype(np.float16))
        osl = slice(k * OSL, (k + 1) * OSL)
        w2c = np.ascontiguousarray(cW2[osl, :].T.astype(np.float16))
        w2p = np.ascontiguousarray(pW2[osl, :].T.astype(np.float16))

        b1big = np.empty((128, 2, NHT, B), np.float32)
        for c, b1 in enumerate((cb1, pb1)):
            b1m = b1.reshape(NHT, 128).T                      # [p, ht]
            b1big[:, c, :, :] = b1m[:, :, None]
        # each core adds the bias before the AllReduce -> divide by NCORE
        # (exact in fp32: NCORE is a power of two)
        b1big = np.ascontiguousarray(b1big.reshape(128, -1)) / np.float32(NCORE)

        b2cols = np.empty((128, 2, NOT), np.float32)
        for c, b2 in enumerate((cb2, pb2)):
            b2cols[:, c, :] = b2[osl].reshape(NOT, 128).T
        b2cols = np.ascontiguousarray(b2cols.reshape(128, -1))

        in_maps.append(dict(
            xs=xs, c0t=c0t, tap=tapm, invn=invn,
            convA=convA, convB=convB,
            w1c=w1c, w1p=w1p, w2c=w2c, w2p=w2p,
            b1big=b1big, b2c=b2cols))
    return in_maps


def assemble_output(results):
    out = np.empty((B, HALF, 2), np.float32)
    for k, r in enumerate(results):
        o = np.asarray(r["out"]).reshape(2, NOT, 128, B)      # [c, ot, p, b]
        for c in range(2):
            for ot in range(NOT):
                out[:, k * OSL + ot * 128:k * OSL + (ot + 1) * 128, c] = o[c, ot].T
    return out


_NC_CACHE = {}


def _get_nc(**kw):
    key = tuple(sorted(kw.items()))
    if key not in _NC_CACHE:
        _NC_CACHE[key] = build_nc(**kw)
    return _NC_CACHE[key]


def run(inputs, trace=False, **build_kw):
    nc = _get_nc(**build_kw)
    in_maps = host_prepare(**inputs)
    res = run_bass_kernel_spmd(nc, in_maps, core_ids=list(range(NCORE)),
                               trace=trace)
    return assemble_output(res.results), res


def kernel(**inputs):
    out, _ = run(inputs)
    return out
